# revision 1
# baseline (speedup 1.0000x reference)
"""GCN (2-layer, GCNConv + log_softmax) on 8 Trainium2 NeuronCores.

Strategy (1D node partition, per sharding hint):
  - Nodes padded to N_PAD = 392*128 and sharded contiguously: 49 blocks of 128
    dst-nodes per core.
  - CPU preprocessing: add self-loops, compute symmetric norm, sort edges by
    dst, pack per (core, block) into fixed-size edge tiles of 128 (padded with
    norm=0 edges so all cores run an identical instruction stream).
  - On device per core:
      GEMM1: h = x_shard @ W1 (PE, bf16 operands, fp32 accum)
      AllGather h -> full h table in local HBM
      Agg1 per dst block: indirect-DMA gather h[src] for all edge tiles of the
        block, build scaled selector S[e,dst] = (seg[e]==dst)*norm[e] on DVE,
        segment-sum via PE matmul accumulation into PSUM [hid, dst];
        relu(agg+b1) on ACT; fused GEMM2 -> h2 block; store to h2 shard.
      AllGather h2 -> full h2 table
      Agg2 per dst block: gather h2[src], same selector, accumulate [dst, cls];
        +b2, log_softmax on DVE/ACT; store output shard.
  - Host concatenates the 8 output shards and strips padding.
"""

import math

import numpy as np
import ml_dtypes

P = 128
NCORES = 8

# Full-problem constants (hardcoded per harness contract).
N_NODES = 50000
N_EDGES = 800000
F_IN = 512
HIDDEN = 128
N_CLASSES = 40

# Runtime-tunable knobs (test.py may override before calling kernel()).
TRACE = False
TRACE_KWARGS = {}
H_DTYPE = "bfloat16"    # dtype of the h (layer-1 projected) gather table
H2_DTYPE = "float32"    # dtype of the h2 (layer-2 projected) gather table
X_DTYPE = "bfloat16"    # GEMM1 operand dtype

LAST_RESULT = {}        # test.py introspection (exec time etc.)


def _np_dt(name):
    return {"float32": np.float32, "bfloat16": ml_dtypes.bfloat16}[name]


# --------------------------------------------------------------------------
# CPU preprocessing
# --------------------------------------------------------------------------

def _preprocess(edge_index, n_nodes, blocks_per_core):
    """Sort edges (plus self-loops) by dst, pack into fixed-count edge tiles.

    Returns (srcs, segs, norms, T):
      srcs  [NCORES, 128, BPC*T] int32   src node id of edge p in tile g
      segs  [NCORES, 128, BPC*T] float32 dst % 128 (local row in block)
      norms [NCORES, 128, BPC*T] float32 dinv[src]*dinv[dst] (0 for padding)
      T = edge tiles per block (uniform across all cores/blocks)
    """
    nblk = NCORES * blocks_per_core
    src = np.asarray(edge_index[0], dtype=np.int64)
    dst = np.asarray(edge_index[1], dtype=np.int64)

    deg = np.bincount(dst, minlength=n_nodes).astype(np.float32) + 1.0
    dinv = (1.0 / np.sqrt(deg)).astype(np.float32)

    loops = np.arange(n_nodes, dtype=np.int64)
    all_src = np.concatenate([src, loops])
    all_dst = np.concatenate([dst, loops])
    norm = dinv[all_src] * dinv[all_dst]

    order = np.argsort(all_dst, kind="stable")
    s_src = all_src[order].astype(np.int32)
    s_dst = all_dst[order]
    s_norm = norm[order].astype(np.float32)

    blk = s_dst // P
    seg = (s_dst % P).astype(np.float32)
    counts = np.bincount(blk, minlength=nblk)
    T = max(1, int(math.ceil(counts.max() / P)))

    nt = blocks_per_core * T
    srcs = np.zeros((NCORES, P, nt), np.int32)
    segs = np.zeros((NCORES, P, nt), np.float32)
    norms = np.zeros((NCORES, P, nt), np.float32)

    starts = np.concatenate([[0], np.cumsum(counts)])
    for b in range(nblk):
        c, bl = divmod(b, blocks_per_core)
        lo, hi = int(starts[b]), int(starts[b + 1])
        n = hi - lo
        if n == 0:
            continue
        j = np.arange(n)
        g = bl * T + j // P
        p = j % P
        srcs[c, p, g] = s_src[lo:hi]
        segs[c, p, g] = seg[lo:hi]
        norms[c, p, g] = s_norm[lo:hi]
    return srcs, segs, norms, T


# --------------------------------------------------------------------------
# Device program
# --------------------------------------------------------------------------

def _build_program(f_in, hidden, ncls_pad, blocks_per_core, T, hdt_name,
                   h2dt_name, xdt_name):
    import concourse.bacc as bacc
    import concourse.bass as bass
    import concourse.mybir as mybir
    import concourse.tile as tile

    dt = mybir.dt
    name2dt = {"float32": dt.float32, "bfloat16": dt.bfloat16}
    hdt = name2dt[hdt_name]
    h2dt = name2dt[h2dt_name]
    xdt = name2dt[xdt_name]
    f32 = dt.float32

    shard = blocks_per_core * P
    n_pad = NCORES * shard
    nt = blocks_per_core * T
    kt = f_in // P  # k-tiles in GEMM1

    nc = bacc.Bacc(
        "TRN2",
        target_bir_lowering=False,
        debug=False,
        enable_asserts=False,
        num_devices=NCORES,
    )

    # Kernel I/O
    xt_d = nc.dram_tensor("xt", [f_in, shard], xdt, kind="ExternalInput")
    w1_d = nc.dram_tensor("w1", [P, kt * hidden], xdt, kind="ExternalInput")
    b1_d = nc.dram_tensor("b1", [P, 1], f32, kind="ExternalInput")
    w2_d = nc.dram_tensor("w2", [hidden, ncls_pad], f32, kind="ExternalInput")
    b2_d = nc.dram_tensor("b2t", [P, ncls_pad], f32, kind="ExternalInput")
    iota_d = nc.dram_tensor("iotaw", [P, T * P], f32, kind="ExternalInput")
    srcs_d = nc.dram_tensor("srcs", [P, nt], dt.int32, kind="ExternalInput")
    segs_d = nc.dram_tensor("segs", [P, nt], f32, kind="ExternalInput")
    norms_d = nc.dram_tensor("norms", [P, nt], f32, kind="ExternalInput")
    out_d = nc.dram_tensor("out", [shard, N_CLASSES], f32, kind="ExternalOutput")

    RG = [list(range(NCORES))]

    with tile.TileContext(nc) as tc:
        with (
            tc.tile_pool(name="const", bufs=1) as const,
            tc.tile_pool(name="dram", bufs=1, space="DRAM") as dram,
            tc.tile_pool(name="sb", bufs=3) as sb,
            tc.tile_pool(name="psum", bufs=2, space="PSUM") as psum,
        ):
            # Internal DRAM buffers
            h_ag_in = dram.tile([shard, hidden], hdt)
            h_full = dram.tile([n_pad, hidden], hdt, addr_space="Shared")
            h2_ag_in = dram.tile([shard, ncls_pad], h2dt)
            h2_full = dram.tile([n_pad, ncls_pad], h2dt, addr_space="Shared")

            # Constants into SBUF
            w1_sb = const.tile([P, kt * hidden], xdt)
            nc.sync.dma_start(out=w1_sb[:], in_=w1_d[:])
            b1_sb = const.tile([P, 1], f32)
            nc.sync.dma_start(out=b1_sb[:], in_=b1_d[:])
            w2_sb = const.tile([hidden, ncls_pad], f32)
            nc.sync.dma_start(out=w2_sb[:], in_=w2_d[:])
            b2_sb = const.tile([P, ncls_pad], f32)
            nc.sync.dma_start(out=b2_sb[:], in_=b2_d[:])
            iota_sb = const.tile([P, T * P], f32)
            nc.sync.dma_start(out=iota_sb[:], in_=iota_d[:])
            srcs_sb = const.tile([P, nt], dt.int32)
            nc.sync.dma_start(out=srcs_sb[:], in_=srcs_d[:])
            segs_sb = const.tile([P, nt], f32)
            nc.sync.dma_start(out=segs_sb[:], in_=segs_d[:])
            norms_sb = const.tile([P, nt], f32)
            nc.sync.dma_start(out=norms_sb[:], in_=norms_d[:])

            # ---------------- Phase 1: GEMM1 (h = x @ W1) ----------------
            for i in range(blocks_per_core):
                psum_h = psum.tile([P, hidden], f32, tag="psum_h")
                for k in range(kt):
                    xt_t = sb.tile([P, P], xdt, tag="xt", bufs=4)
                    nc.sync.dma_start(
                        out=xt_t[:],
                        in_=xt_d[k * P:(k + 1) * P, i * P:(i + 1) * P],
                    )
                    nc.tensor.matmul(
                        out=psum_h[:],
                        lhsT=xt_t[:],
                        rhs=w1_sb[:, k * hidden:(k + 1) * hidden],
                        start=(k == 0),
                        stop=(k == kt - 1),
                    )
                h_t = sb.tile([P, hidden], hdt, tag="h_t")
                nc.vector.tensor_copy(out=h_t[:], in_=psum_h[:])
                nc.sync.dma_start(
                    out=h_ag_in[i * P:(i + 1) * P, :], in_=h_t[:]
                )

            # ---------------- AllGather h ----------------
            nc.gpsimd.collective_compute(
                "AllGather",
                mybir.AluOpType.bypass,
                replica_groups=RG,
                ins=[h_ag_in[:]],
                outs=[h_full[:]],
            )

            # ---------------- Phase 2: Agg1 + relu + GEMM2 ----------------
            def build_selector(b, seldt):
                g0 = b * T
                sel = sb.tile([P, T * P], seldt, tag="sel")
                sel3 = sel[:].rearrange("p (t d) -> p t d", d=P)
                nc.vector.tensor_tensor(
                    out=sel3,
                    in0=iota_sb[:].rearrange("p (t d) -> p t d", d=P),
                    in1=segs_sb[:, g0:g0 + T].to_broadcast([P, T, P]),
                    op=mybir.AluOpType.is_equal,
                )
                nc.vector.tensor_tensor(
                    out=sel3,
                    in0=sel3,
                    in1=norms_sb[:, g0:g0 + T].to_broadcast([P, T, P]),
                    op=mybir.AluOpType.mult,
                )
                return sel

            for b in range(blocks_per_core):
                g0 = b * T
                msg = sb.tile([P, T * hidden], hdt, tag="msg")
                for t in range(T):
                    nc.gpsimd.indirect_dma_start(
                        out=msg[:, t * hidden:(t + 1) * hidden],
                        out_offset=None,
                        in_=h_full[:],
                        in_offset=bass.IndirectOffsetOnAxis(
                            ap=srcs_sb[:, g0 + t:g0 + t + 1], axis=0
                        ),
                    )
                sel = build_selector(b, hdt)
                psum1 = psum.tile([P, P], f32, tag="psum1")
                for t in range(T):
                    nc.tensor.matmul(
                        out=psum1[:],
                        lhsT=msg[:, t * hidden:(t + 1) * hidden],
                        rhs=sel[:, t * P:(t + 1) * P],
                        start=(t == 0),
                        stop=(t == T - 1),
                    )
                # psum1 = agg1^T : [hidden, dst]; relu(agg + b1) with b1 along
                # partitions.
                a1 = sb.tile([P, P], f32, tag="a1")
                nc.scalar.activation(
                    out=a1[:], in_=psum1[:],
                    func=mybir.ActivationFunctionType.Relu,
                    bias=b1_sb[:, 0:1],
                )
                psum2 = psum.tile([P, ncls_pad], f32, tag="psum2")
                nc.tensor.matmul(
                    out=psum2[:], lhsT=a1[:], rhs=w2_sb[:],
                    start=True, stop=True,
                )
                h2_t = sb.tile([P, ncls_pad], h2dt, tag="h2_t")
                nc.vector.tensor_copy(out=h2_t[:], in_=psum2[:])
                nc.sync.dma_start(
                    out=h2_ag_in[b * P:(b + 1) * P, :], in_=h2_t[:]
                )

            # ---------------- AllGather h2 ----------------
            nc.gpsimd.collective_compute(
                "AllGather",
                mybir.AluOpType.bypass,
                replica_groups=RG,
                ins=[h2_ag_in[:]],
                outs=[h2_full[:]],
            )

            # ---------------- Phase 3: Agg2 + bias + log_softmax ----------------
            for b in range(blocks_per_core):
                g0 = b * T
                msg2 = sb.tile([P, T * ncls_pad], h2dt, tag="msg2")
                for t in range(T):
                    nc.gpsimd.indirect_dma_start(
                        out=msg2[:, t * ncls_pad:(t + 1) * ncls_pad],
                        out_offset=None,
                        in_=h2_full[:],
                        in_offset=bass.IndirectOffsetOnAxis(
                            ap=srcs_sb[:, g0 + t:g0 + t + 1], axis=0
                        ),
                    )
                sel = build_selector(b, h2dt)
                psum_o = psum.tile([P, ncls_pad], f32, tag="psum_o")
                for t in range(T):
                    nc.tensor.matmul(
                        out=psum_o[:],
                        lhsT=sel[:, t * P:(t + 1) * P],
                        rhs=msg2[:, t * ncls_pad:(t + 1) * ncls_pad],
                        start=(t == 0),
                        stop=(t == T - 1),
                    )
                logits = sb.tile([P, N_CLASSES], f32, tag="logits")
                nc.vector.tensor_tensor(
                    out=logits[:], in0=psum_o[:, 0:N_CLASSES],
                    in1=b2_sb[:, 0:N_CLASSES], op=mybir.AluOpType.add,
                )
                negm = sb.tile([P, 1], f32, tag="negm")
                nc.vector.reduce_max(
                    out=negm[:], in_=logits[:], axis=mybir.AxisListType.X
                )
                nc.vector.tensor_scalar_mul(
                    out=negm[:], in0=negm[:], scalar1=-1.0
                )
                expv = sb.tile([P, N_CLASSES], f32, tag="expv")
                nc.scalar.activation(
                    out=expv[:], in_=logits[:],
                    func=mybir.ActivationFunctionType.Exp,
                    bias=negm[:, 0:1],
                )
                ssum = sb.tile([P, 1], f32, tag="ssum")
                nc.vector.reduce_sum(
                    out=ssum[:], in_=expv[:], axis=mybir.AxisListType.X
                )
                lns = sb.tile([P, 1], f32, tag="lns")
                nc.scalar.activation(
                    out=lns[:], in_=ssum[:],
                    func=mybir.ActivationFunctionType.Ln,
                )
                outt = sb.tile([P, N_CLASSES], f32, tag="outt")
                nc.vector.tensor_scalar(
                    out=outt[:], in0=logits[:],
                    scalar1=negm[:, 0:1], scalar2=lns[:, 0:1],
                    op0=mybir.AluOpType.add, op1=mybir.AluOpType.subtract,
                )
                nc.sync.dma_start(
                    out=out_d[b * P:(b + 1) * P, :], in_=outt[:]
                )

    nc.compile()
    return nc


# --------------------------------------------------------------------------
# Host orchestration
# --------------------------------------------------------------------------

def _run(x, edge_index, W1, b1, W2, b2, blocks_per_core):
    from concourse.bass_utils import run_bass_kernel_spmd

    global LAST_RESULT

    x = np.asarray(x, dtype=np.float32)
    W1 = np.asarray(W1, dtype=np.float32)
    b1v = np.asarray(b1, dtype=np.float32).reshape(-1)
    W2 = np.asarray(W2, dtype=np.float32)
    b2v = np.asarray(b2, dtype=np.float32).reshape(-1)

    n_nodes, f_in = x.shape
    hidden = W1.shape[1]
    ncls = W2.shape[1]
    ncls_pad = 64 if ncls <= 64 else int(math.ceil(ncls / P) * P)
    assert hidden == P and ncls == N_CLASSES

    shard = blocks_per_core * P
    n_pad = NCORES * shard
    assert n_pad >= n_nodes

    srcs, segs, norms, T = _preprocess(edge_index, n_nodes, blocks_per_core)

    nc = _build_program(
        f_in, hidden, ncls_pad, blocks_per_core, T,
        H_DTYPE, H2_DTYPE, X_DTYPE,
    )

    xdt_np = _np_dt(X_DTYPE)
    kt = f_in // P

    x_pad = np.zeros((n_pad, f_in), np.float32)
    x_pad[:n_nodes] = x
    w1r = np.ascontiguousarray(
        W1.reshape(kt, P, hidden).transpose(1, 0, 2).reshape(P, kt * hidden)
    ).astype(xdt_np)
    w2p = np.zeros((hidden, ncls_pad), np.float32)
    w2p[:, :ncls] = W2
    b2t = np.zeros((P, ncls_pad), np.float32)
    b2t[:, :ncls] = b2v[None, :]
    iotaw = np.ascontiguousarray(
        np.broadcast_to(
            np.tile(np.arange(P, dtype=np.float32), T), (P, T * P)
        )
    )

    in_maps = []
    for c in range(NCORES):
        xt_c = np.ascontiguousarray(
            x_pad[c * shard:(c + 1) * shard].T
        ).astype(xdt_np)
        in_maps.append({
            "xt": xt_c,
            "w1": w1r,
            "b1": b1v.reshape(P, 1).copy(),
            "w2": w2p,
            "b2t": b2t,
            "iotaw": iotaw,
            "srcs": np.ascontiguousarray(srcs[c]),
            "segs": np.ascontiguousarray(segs[c]),
            "norms": np.ascontiguousarray(norms[c]),
        })

    res = run_bass_kernel_spmd(
        nc, in_maps, core_ids=list(range(NCORES)),
        trace=TRACE, trace_kwargs=dict(TRACE_KWARGS),
    )
    LAST_RESULT = {
        "exec_time_ns": res.exec_time_ns,
        "mean_exec_time_ns": res.mean_exec_time_ns,
        "instructions_and_trace": res.instructions_and_trace,
        "profile_json": res.profile_json,
        "T": T,
        "nc": nc,
        "in_maps": in_maps,
    }
    out = np.concatenate([r["out"] for r in res.results], axis=0)
    return out[:n_nodes]


def kernel(x, edge_index, W1, b1, W2, b2):
    n_nodes = np.asarray(x).shape[0]
    blocks_per_core = int(math.ceil(n_nodes / (NCORES * P)))
    return _run(x, edge_index, W1, b1, W2, b2, blocks_per_core)



# revision 7
# speedup vs baseline: 3.0839x; 3.0839x over previous
"""GCN (2-layer, GCNConv + log_softmax) on 8 Trainium2 NeuronCores.

Strategy (1D node partition, per sharding hint):
  - Nodes padded to N_PAD = 392*128 and sharded contiguously: 49 blocks of 128
    dst-nodes per core.
  - CPU preprocessing: add self-loops, compute symmetric norm, sort edges by
    dst, pack per (core, block) into fixed-size edge tiles of 128 (padded with
    seg=255 / norm=0 edges so all cores run an identical instruction stream).
  - On device per core:
      GEMM1: h = x_shard @ W1 (PE, bf16, 4 big xt strip loads)
      AllGather h -> full h table in local HBM (bf16)
      Agg1 per dst block: ONE batched indirect-DMA gather of h[src] for a
        group of blocks, selector S[e,dst] = (seg[e]==dst)*norm[e] on DVE,
        segment-sum via PE matmul accumulation into PSUM [hid, dst];
        relu(agg+b1) on ACT; fused GEMM2 -> h2 block; scale row v by dinv[v]
        (folds the src-side norm factor of layer 2 into the table) -> h2 bf16.
      AllGather h2 (bf16) -> full h2 table
      Agg2 per dst block: batched gather h2[src], one-hot selector
        (is_equal only; src norm factor is in the table), accumulate
        [dst, cls]; scale by dinv[dst]; +b2; batched log_softmax; one store.
  - Host concatenates the 8 output shards and strips padding.
"""

import math

import numpy as np
import ml_dtypes

P = 128
NCORES = 8

# Full-problem constants (hardcoded per harness contract).
N_NODES = 50000
N_EDGES = 800000
F_IN = 512
HIDDEN = 128
N_CLASSES = 40

NCLS_PAD = 64
GATHER_GROUP = 7          # dst blocks per indirect-DMA gather

# Runtime-tunable knobs (test.py may override before calling kernel()).
TRACE = False
TRACE_KWARGS = {}

LAST_RESULT = {}          # test.py introspection (exec time etc.)

BF16 = ml_dtypes.bfloat16


# --------------------------------------------------------------------------
# CPU preprocessing
# --------------------------------------------------------------------------

def _preprocess(edge_index, n_nodes, blocks_per_core):
    """Sort edges (plus self-loops) by dst, pack into fixed-count edge tiles.

    Returns (srcs, segs, norms, dinv_pad, T):
      srcs  [NCORES, 128, BPC*T] int32   src node id of edge p in tile g
      segs  [NCORES, 128, BPC*T] bf16    dst % 128 (255 for padding slots)
      norms [NCORES, 128, BPC*T] bf16    dinv[src]*dinv[dst] (0 for padding)
      dinv_pad [N_PAD] f32               dinv per node (0 for padding nodes)
      T = edge tiles per block (uniform across all cores/blocks)
    """
    nblk = NCORES * blocks_per_core
    n_pad = nblk * P
    src = np.asarray(edge_index[0], dtype=np.int64)
    dst = np.asarray(edge_index[1], dtype=np.int64)

    deg = np.bincount(dst, minlength=n_nodes).astype(np.float32) + 1.0
    dinv = (1.0 / np.sqrt(deg)).astype(np.float32)

    loops = np.arange(n_nodes, dtype=np.int64)
    all_src = np.concatenate([src, loops])
    all_dst = np.concatenate([dst, loops])
    norm = dinv[all_src] * dinv[all_dst]

    order = np.argsort(all_dst, kind="stable")
    s_src = all_src[order].astype(np.int32)
    s_dst = all_dst[order]
    s_norm = norm[order].astype(np.float32)

    blk = s_dst // P
    seg = (s_dst % P).astype(np.float32)
    counts = np.bincount(blk, minlength=nblk)
    T = max(1, int(math.ceil(counts.max() / P)))

    nt = blocks_per_core * T
    srcs = np.zeros((NCORES, P, nt), np.int32)
    segs = np.full((NCORES, P, nt), 255.0, np.float32)
    norms = np.zeros((NCORES, P, nt), np.float32)

    starts = np.concatenate([[0], np.cumsum(counts)])
    for b in range(nblk):
        c, bl = divmod(b, blocks_per_core)
        lo, hi = int(starts[b]), int(starts[b + 1])
        n = hi - lo
        if n == 0:
            continue
        j = np.arange(n)
        g = bl * T + j // P
        p = j % P
        srcs[c, p, g] = s_src[lo:hi]
        segs[c, p, g] = seg[lo:hi]
        norms[c, p, g] = s_norm[lo:hi]

    dinv_pad = np.zeros((n_pad,), np.float32)
    dinv_pad[:n_nodes] = dinv
    return srcs, segs.astype(BF16), norms.astype(BF16), dinv_pad, T


# --------------------------------------------------------------------------
# Device program
# --------------------------------------------------------------------------

def _build_program(f_in, hidden, ncls_pad, bpc, T):
    import concourse.bacc as bacc
    import concourse.bass as bass
    import concourse.mybir as mybir
    import concourse.tile as tile

    dt = mybir.dt
    bf16 = dt.bfloat16
    f32 = dt.float32

    shard = bpc * P
    n_pad = NCORES * shard
    nt = bpc * T
    kt = f_in // P  # k-tiles in GEMM1
    G = GATHER_GROUP
    ngrp = bpc // G
    assert ngrp * G == bpc

    nc = bacc.Bacc(
        "TRN2",
        target_bir_lowering=False,
        debug=False,
        enable_asserts=False,
        num_devices=NCORES,
    )

    # Kernel I/O
    xt_d = nc.dram_tensor("xt", [f_in, shard], bf16, kind="ExternalInput")
    w1_d = nc.dram_tensor("w1", [P, kt * hidden], bf16, kind="ExternalInput")
    b1_d = nc.dram_tensor("b1", [P, 1], f32, kind="ExternalInput")
    w2_d = nc.dram_tensor("w2", [hidden, ncls_pad], bf16, kind="ExternalInput")
    b2w_d = nc.dram_tensor("b2w", [P, bpc * ncls_pad], f32, kind="ExternalInput")
    iota_d = nc.dram_tensor("iotaw", [P, T * P], bf16, kind="ExternalInput")
    srcs_d = nc.dram_tensor("srcs", [P, nt], dt.int32, kind="ExternalInput")
    segs_d = nc.dram_tensor("segs", [P, nt], bf16, kind="ExternalInput")
    norms_d = nc.dram_tensor("norms", [P, nt], bf16, kind="ExternalInput")
    dinvn_d = nc.dram_tensor("dinvn", [P, bpc], f32, kind="ExternalInput")
    out_d = nc.dram_tensor("out", [shard, N_CLASSES], f32, kind="ExternalOutput")

    RG = [list(range(NCORES))]
    AF = mybir.ActivationFunctionType

    with tile.TileContext(nc) as tc:
        with (
            tc.tile_pool(name="const", bufs=1) as const,
            tc.tile_pool(name="dram", bufs=1, space="DRAM") as dram,
            tc.tile_pool(name="sb", bufs=1) as sb,
            tc.tile_pool(name="psum", bufs=2, space="PSUM") as psum,
        ):
            # Internal DRAM buffers
            h_ag_in = dram.tile([shard, hidden], bf16)
            h_full = dram.tile([n_pad, hidden], bf16, addr_space="Shared")
            h2_ag_in = dram.tile([shard, ncls_pad], bf16)
            h2_full = dram.tile([n_pad, ncls_pad], bf16, addr_space="Shared")

            # Constants into SBUF
            w1_sb = const.tile([P, kt * hidden], bf16)
            nc.sync.dma_start(out=w1_sb[:], in_=w1_d[:])
            b1_sb = const.tile([P, 1], f32)
            nc.sync.dma_start(out=b1_sb[:], in_=b1_d[:])
            w2_sb = const.tile([hidden, ncls_pad], bf16)
            nc.sync.dma_start(out=w2_sb[:], in_=w2_d[:])
            b2w_sb = const.tile([P, bpc * ncls_pad], f32)
            nc.sync.dma_start(out=b2w_sb[:], in_=b2w_d[:])
            iota_sb = const.tile([P, T * P], bf16)
            nc.sync.dma_start(out=iota_sb[:], in_=iota_d[:])
            srcs_sb = const.tile([P, nt], dt.int32)
            nc.sync.dma_start(out=srcs_sb[:], in_=srcs_d[:])
            segs_sb = const.tile([P, nt], bf16)
            nc.sync.dma_start(out=segs_sb[:], in_=segs_d[:])
            norms_sb = const.tile([P, nt], bf16)
            nc.sync.dma_start(out=norms_sb[:], in_=norms_d[:])
            dinvn_sb = const.tile([P, bpc], f32)
            nc.sync.dma_start(out=dinvn_sb[:], in_=dinvn_d[:])

            # Persistent big SBUF staging tiles
            h_big = sb.tile([P, bpc * hidden], bf16, tag="h_big", bufs=1)
            h2_big = sb.tile([P, bpc * ncls_pad], bf16, tag="h2_big", bufs=1)
            lg_big = sb.tile([P, bpc * ncls_pad], f32, tag="lg_big", bufs=1)
            expv = sb.tile([P, bpc * N_CLASSES], f32, tag="expv", bufs=1)
            out_big = sb.tile([P, bpc * N_CLASSES], f32, tag="out_big", bufs=1)
            maxs = sb.tile([P, bpc], f32, tag="maxs", bufs=1)
            sums = sb.tile([P, bpc], f32, tag="sums", bufs=1)
            lns = sb.tile([P, bpc], f32, tag="lns", bufs=1)

            iota3 = iota_sb[:].rearrange("p (t d) -> p t d", d=P)

            # ---------------- Phase 1: GEMM1 (h = x @ W1) ----------------
            with tc.tile_pool(name="xtp", bufs=1) as xtp:
                xt_sb = xtp.tile([P, kt * shard], bf16, tag="xt", bufs=1)
                for k in range(kt):
                    nc.sync.dma_start(
                        out=xt_sb[:, k * shard:(k + 1) * shard],
                        in_=xt_d[k * P:(k + 1) * P, :],
                    )
                for i in range(bpc):
                    psum_h = psum.tile([P, P], f32, tag="pmm")
                    for k in range(kt):
                        nc.tensor.matmul(
                            out=psum_h[:],
                            lhsT=xt_sb[:, k * shard + i * P:k * shard + (i + 1) * P],
                            rhs=w1_sb[:, k * hidden:(k + 1) * hidden],
                            start=(k == 0),
                            stop=(k == kt - 1),
                        )
                    nc.vector.tensor_copy(
                        out=h_big[:, i * hidden:(i + 1) * hidden], in_=psum_h[:]
                    )
                nc.sync.dma_start(
                    out=h_ag_in[:].rearrange("(t p) f -> p t f", p=P),
                    in_=h_big[:].rearrange("p (t f) -> p t f", f=hidden),
                )

            # ---------------- AllGather h ----------------
            nc.gpsimd.collective_compute(
                "AllGather",
                mybir.AluOpType.bypass,
                replica_groups=RG,
                ins=[h_ag_in[:]],
                outs=[h_full[:]],
            )

            # ---------------- Phase 2: Agg1 + relu + GEMM2 ----------------
            with tc.tile_pool(name="msgp", bufs=1) as msgp:
                for g in range(ngrp):
                    msg = msgp.tile([P, G * T * hidden], bf16, tag="msg", bufs=2)
                    nc.gpsimd.indirect_dma_start(
                        out=msg[:],
                        out_offset=None,
                        in_=h_full[:],
                        in_offset=bass.IndirectOffsetOnAxis(
                            ap=srcs_sb[:, g * G * T:(g + 1) * G * T], axis=0
                        ),
                    )
                    for j in range(G):
                        b = g * G + j
                        g0 = b * T
                        sel = sb.tile([P, T * P], bf16, tag="sel", bufs=4)
                        sel3 = sel[:].rearrange("p (t d) -> p t d", d=P)
                        nc.vector.tensor_tensor(
                            out=sel3,
                            in0=iota3,
                            in1=segs_sb[:, g0:g0 + T].to_broadcast([P, T, P]),
                            op=mybir.AluOpType.is_equal,
                        )
                        nc.vector.tensor_tensor(
                            out=sel3,
                            in0=sel3,
                            in1=norms_sb[:, g0:g0 + T].to_broadcast([P, T, P]),
                            op=mybir.AluOpType.mult,
                        )
                        psum1 = psum.tile([P, P], f32, tag="pmm")
                        for t in range(T):
                            nc.tensor.matmul(
                                out=psum1[:],
                                lhsT=msg[:, (j * T + t) * hidden:(j * T + t + 1) * hidden],
                                rhs=sel[:, t * P:(t + 1) * P],
                                start=(t == 0),
                                stop=(t == T - 1),
                            )
                        a1 = sb.tile([P, P], bf16, tag="a1", bufs=3)
                        nc.scalar.activation(
                            out=a1[:], in_=psum1[:],
                            func=AF.Relu,
                            bias=b1_sb[:, 0:1],
                        )
                        psum2 = psum.tile([P, ncls_pad], f32, tag="pcl")
                        nc.tensor.matmul(
                            out=psum2[:], lhsT=a1[:], rhs=w2_sb[:],
                            start=True, stop=True,
                        )
                        # h2 row v scaled by dinv[v]: folds layer-2 src norm
                        # into the gather table.
                        nc.scalar.activation(
                            out=h2_big[:, b * ncls_pad:(b + 1) * ncls_pad],
                            in_=psum2[:],
                            func=AF.Copy,
                            scale=dinvn_sb[:, b:b + 1],
                        )
                nc.sync.dma_start(
                    out=h2_ag_in[:].rearrange("(t p) f -> p t f", p=P),
                    in_=h2_big[:].rearrange("p (t f) -> p t f", f=ncls_pad),
                )

            # ---------------- AllGather h2 ----------------
            nc.gpsimd.collective_compute(
                "AllGather",
                mybir.AluOpType.bypass,
                replica_groups=RG,
                ins=[h2_ag_in[:]],
                outs=[h2_full[:]],
            )

            # ---------------- Phase 3: Agg2 ----------------
            with tc.tile_pool(name="msg2p", bufs=1) as msg2p:
                for g in range(ngrp):
                    msg2 = msg2p.tile([P, G * T * ncls_pad], bf16, tag="msg2",
                                      bufs=2)
                    nc.gpsimd.indirect_dma_start(
                        out=msg2[:],
                        out_offset=None,
                        in_=h2_full[:],
                        in_offset=bass.IndirectOffsetOnAxis(
                            ap=srcs_sb[:, g * G * T:(g + 1) * G * T], axis=0
                        ),
                    )
                    for j in range(G):
                        b = g * G + j
                        g0 = b * T
                        sel = sb.tile([P, T * P], bf16, tag="sel3", bufs=4)
                        sel3 = sel[:].rearrange("p (t d) -> p t d", d=P)
                        nc.vector.tensor_tensor(
                            out=sel3,
                            in0=iota3,
                            in1=segs_sb[:, g0:g0 + T].to_broadcast([P, T, P]),
                            op=mybir.AluOpType.is_equal,
                        )
                        psum_o = psum.tile([P, ncls_pad], f32, tag="pcl")
                        for t in range(T):
                            nc.tensor.matmul(
                                out=psum_o[:],
                                lhsT=sel[:, t * P:(t + 1) * P],
                                rhs=msg2[:, (j * T + t) * ncls_pad:(j * T + t + 1) * ncls_pad],
                                start=(t == 0),
                                stop=(t == T - 1),
                            )
                        nc.vector.tensor_scalar_mul(
                            out=lg_big[:, b * ncls_pad:(b + 1) * ncls_pad],
                            in0=psum_o[:],
                            scalar1=dinvn_sb[:, b:b + 1],
                        )

            # ------------- bias + batched log_softmax + store -------------
            nc.vector.tensor_tensor(
                out=lg_big[:], in0=lg_big[:], in1=b2w_sb[:],
                op=mybir.AluOpType.add,
            )
            l40 = lg_big[:].rearrange(
                "p (t f) -> p t f", f=ncls_pad)[:, :, 0:N_CLASSES]
            nc.vector.tensor_reduce(
                out=maxs[:], in_=l40, axis=mybir.AxisListType.X,
                op=mybir.AluOpType.max,
            )
            nc.vector.tensor_tensor(
                out=l40, in0=l40,
                in1=maxs[:].to_broadcast([P, bpc, N_CLASSES]),
                op=mybir.AluOpType.subtract,
            )
            expv3 = expv[:].rearrange("p (t f) -> p t f", f=N_CLASSES)
            nc.scalar.activation(out=expv3, in_=l40, func=AF.Exp)
            nc.vector.tensor_reduce(
                out=sums[:], in_=expv3, axis=mybir.AxisListType.X,
                op=mybir.AluOpType.add,
            )
            nc.scalar.activation(out=lns[:], in_=sums[:], func=AF.Ln)
            nc.vector.tensor_tensor(
                out=out_big[:].rearrange("p (t f) -> p t f", f=N_CLASSES),
                in0=l40,
                in1=lns[:].to_broadcast([P, bpc, N_CLASSES]),
                op=mybir.AluOpType.subtract,
            )
            nc.sync.dma_start(
                out=out_d[:].rearrange("(t p) f -> p t f", p=P),
                in_=out_big[:].rearrange("p (t f) -> p t f", f=N_CLASSES),
            )

    nc.compile()
    return nc


# --------------------------------------------------------------------------
# Host orchestration
# --------------------------------------------------------------------------

def _prepare(x, edge_index, W1, b1, W2, b2, bpc):
    x = np.asarray(x, dtype=np.float32)
    W1 = np.asarray(W1, dtype=np.float32)
    b1v = np.asarray(b1, dtype=np.float32).reshape(-1)
    W2 = np.asarray(W2, dtype=np.float32)
    b2v = np.asarray(b2, dtype=np.float32).reshape(-1)

    n_nodes, f_in = x.shape
    hidden = W1.shape[1]
    ncls = W2.shape[1]
    assert hidden == P and ncls == N_CLASSES

    shard = bpc * P
    n_pad = NCORES * shard
    assert n_pad >= n_nodes

    srcs, segs, norms, dinv_pad, T = _preprocess(edge_index, n_nodes, bpc)

    nc = _build_program(f_in, hidden, NCLS_PAD, bpc, T)

    kt = f_in // P

    x_pad = np.zeros((n_pad, f_in), np.float32)
    x_pad[:n_nodes] = x
    w1r = np.ascontiguousarray(
        W1.reshape(kt, P, hidden).transpose(1, 0, 2).reshape(P, kt * hidden)
    ).astype(BF16)
    w2p = np.zeros((hidden, NCLS_PAD), np.float32)
    w2p[:, :ncls] = W2
    w2p = w2p.astype(BF16)
    b2blk = np.zeros((P, NCLS_PAD), np.float32)
    b2blk[:, :ncls] = b2v[None, :]
    b2w = np.tile(b2blk, (1, bpc))
    iotaw = np.ascontiguousarray(
        np.broadcast_to(
            np.tile(np.arange(P, dtype=np.float32), T), (P, T * P)
        )
    ).astype(BF16)
    dinv_cores = dinv_pad.reshape(NCORES, bpc, P)

    in_maps = []
    for c in range(NCORES):
        xt_c = np.ascontiguousarray(
            x_pad[c * shard:(c + 1) * shard].T
        ).astype(BF16)
        in_maps.append({
            "xt": xt_c,
            "w1": w1r,
            "b1": b1v.reshape(P, 1).copy(),
            "w2": w2p,
            "b2w": b2w,
            "iotaw": iotaw,
            "srcs": np.ascontiguousarray(srcs[c]),
            "segs": np.ascontiguousarray(segs[c]),
            "norms": np.ascontiguousarray(norms[c]),
            "dinvn": np.ascontiguousarray(dinv_cores[c].T),
        })
    return nc, in_maps, T


def _run(x, edge_index, W1, b1, W2, b2, bpc):
    from concourse.bass_utils import run_bass_kernel_spmd

    global LAST_RESULT

    n_nodes = np.asarray(x).shape[0]
    nc, in_maps, T = _prepare(x, edge_index, W1, b1, W2, b2, bpc)

    res = run_bass_kernel_spmd(
        nc, in_maps, core_ids=list(range(NCORES)),
        trace=TRACE, trace_kwargs=dict(TRACE_KWARGS),
    )
    LAST_RESULT = {
        "exec_time_ns": res.exec_time_ns,
        "mean_exec_time_ns": res.mean_exec_time_ns,
        "instructions_and_trace": res.instructions_and_trace,
        "profile_json": res.profile_json,
        "T": T,
        "nc": nc,
        "in_maps": in_maps,
    }
    out = np.concatenate([r["out"] for r in res.results], axis=0)
    return out[:n_nodes]


def kernel(x, edge_index, W1, b1, W2, b2):
    n_nodes = np.asarray(x).shape[0]
    bpc = int(math.ceil(n_nodes / (NCORES * P)))
    return _run(x, edge_index, W1, b1, W2, b2, bpc)


# revision 16
# speedup vs baseline: 4.8022x; 1.5572x over previous
"""GCN (2-layer, GCNConv + log_softmax) on 8 Trainium2 NeuronCores.

Strategy (1D node partition, per sharding hint):
  - Nodes padded to N_PAD = 392*128 and sharded contiguously: 49 blocks of 128
    dst-nodes per core.
  - CPU preprocessing: add self-loops, compute symmetric norm, sort edges by
    dst, pack per (core, block) into fixed-size edge tiles of 128 (padded with
    seg=255 / norm=0 edges so all cores run an identical instruction stream).
  - On device per core:
      GEMM1: h = x_shard @ W1 (PE, bf16, 4 big xt strip loads)
      AllGather h -> full h table in local HBM (bf16)
      Agg1 per dst block: ONE batched indirect-DMA gather of h[src] for a
        group of blocks, selector S[e,dst] = (seg[e]==dst)*norm[e] on DVE,
        segment-sum via PE matmul accumulation into PSUM [hid, dst];
        relu(agg+b1) on ACT; fused GEMM2 -> h2 block; scale row v by dinv[v]
        (folds the src-side norm factor of layer 2 into the table) -> h2 bf16.
      AllGather h2 (bf16) -> full h2 table
      Agg2 per dst block: batched gather h2[src], one-hot selector
        (is_equal only; src norm factor is in the table), accumulate
        [dst, cls]; scale by dinv[dst]; +b2; batched log_softmax; one store.
  - Host concatenates the 8 output shards and strips padding.
"""

import math

import numpy as np
import ml_dtypes

P = 128
NCORES = 8

# Full-problem constants (hardcoded per harness contract).
N_NODES = 50000
N_EDGES = 800000
F_IN = 512
HIDDEN = 128
N_CLASSES = 40

NCLS_PAD = 64
GATHER_GROUP = 7          # dst blocks per indirect-DMA gather

# Runtime-tunable knobs (test.py may override before calling kernel()).
TRACE = False
TRACE_KWARGS = {}
# Ablation: how much of the pipeline to run ("p1","ag1","p2","ag2","full").
ABLATE = "full"

LAST_RESULT = {}          # test.py introspection (exec time etc.)

BF16 = ml_dtypes.bfloat16


# --------------------------------------------------------------------------
# CPU preprocessing
# --------------------------------------------------------------------------

def _preprocess(edge_index, n_nodes, blocks_per_core):
    """Sort edges (plus self-loops) by dst, pack into fixed-count edge tiles.

    Returns (srcs, segs, norms, dinv_pad, T):
      srcs  [NCORES, 128, BPC*T] int32   src node id of edge p in tile g
      segs  [NCORES, 128, BPC*T] bf16    dst % 128 (255 for padding slots)
      norms [NCORES, 128, BPC*T] bf16    dinv[src]*dinv[dst] (0 for padding)
      dinv_pad [N_PAD] f32               dinv per node (0 for padding nodes)
      T = edge tiles per block (uniform across all cores/blocks)
    """
    nblk = NCORES * blocks_per_core
    n_pad = nblk * P
    src = np.asarray(edge_index[0], dtype=np.int64)
    dst = np.asarray(edge_index[1], dtype=np.int64)

    deg = np.bincount(dst, minlength=n_nodes).astype(np.float32) + 1.0
    dinv = (1.0 / np.sqrt(deg)).astype(np.float32)

    loops = np.arange(n_nodes, dtype=np.int64)
    all_src = np.concatenate([src, loops])
    all_dst = np.concatenate([dst, loops])
    norm = dinv[all_src] * dinv[all_dst]

    order = np.argsort(all_dst, kind="stable")
    s_src = all_src[order].astype(np.int32)
    s_dst = all_dst[order]
    s_norm = norm[order].astype(np.float32)

    blk = s_dst // P
    seg = (s_dst % P).astype(np.float32)
    counts = np.bincount(blk, minlength=nblk)
    T = max(1, int(math.ceil(counts.max() / P)))

    nt = blocks_per_core * T
    srcs = np.zeros((NCORES, P, nt), np.int32)
    segs = np.full((NCORES, P, nt), 255.0, np.float32)
    norms = np.zeros((NCORES, P, nt), np.float32)

    starts = np.concatenate([[0], np.cumsum(counts)])
    for b in range(nblk):
        c, bl = divmod(b, blocks_per_core)
        lo, hi = int(starts[b]), int(starts[b + 1])
        n = hi - lo
        if n == 0:
            continue
        j = np.arange(n)
        g = bl * T + j // P
        p = j % P
        srcs[c, p, g] = s_src[lo:hi]
        segs[c, p, g] = seg[lo:hi]
        norms[c, p, g] = s_norm[lo:hi]

    dinv_pad = np.zeros((n_pad,), np.float32)
    dinv_pad[:n_nodes] = dinv
    return srcs, segs.astype(BF16), norms.astype(BF16), dinv_pad, T


# --------------------------------------------------------------------------
# Device program
# --------------------------------------------------------------------------

def _build_program(f_in, hidden, ncls_pad, bpc, T):
    import concourse.bacc as bacc
    import concourse.bass as bass
    import concourse.mybir as mybir
    import concourse.tile as tile

    dt = mybir.dt
    bf16 = dt.bfloat16
    f32 = dt.float32

    shard = bpc * P
    n_pad = NCORES * shard
    nt = bpc * T
    kt = f_in // P  # k-tiles in GEMM1
    G = GATHER_GROUP
    ngrp = bpc // G
    assert ngrp * G == bpc

    nc = bacc.Bacc(
        "TRN2",
        target_bir_lowering=False,
        debug=False,
        enable_asserts=False,
        num_devices=NCORES,
    )

    # Kernel I/O
    xt_d = nc.dram_tensor("xt", [f_in, shard], bf16, kind="ExternalInput")
    w1_d = nc.dram_tensor("w1", [P, kt * hidden], bf16, kind="ExternalInput")
    b1_d = nc.dram_tensor("b1", [P, 1], f32, kind="ExternalInput")
    w2_d = nc.dram_tensor("w2", [hidden, ncls_pad], f32, kind="ExternalInput")
    b2w_d = nc.dram_tensor("b2w", [P, bpc * ncls_pad], f32, kind="ExternalInput")
    iota_d = nc.dram_tensor("iotaw", [P, T * P], bf16, kind="ExternalInput")
    srcs_d = nc.dram_tensor("srcs", [P, nt], dt.int32, kind="ExternalInput")
    segs_d = nc.dram_tensor("segs", [P, nt], bf16, kind="ExternalInput")
    norms_d = nc.dram_tensor("norms", [P, nt], bf16, kind="ExternalInput")
    dinvn_d = nc.dram_tensor("dinvn", [P, bpc], f32, kind="ExternalInput")
    out_d = nc.dram_tensor("out", [shard, N_CLASSES], f32, kind="ExternalOutput")

    RG = [list(range(NCORES))]
    AF = mybir.ActivationFunctionType
    lvl = ["p1", "ag1", "p2", "ag2", "full"].index(ABLATE)

    with tile.TileContext(nc) as tc:
        with (
            tc.tile_pool(name="const", bufs=1) as const,
            tc.tile_pool(name="dram", bufs=1, space="DRAM") as dram,
            tc.tile_pool(name="sb", bufs=1) as sb,
            tc.tile_pool(name="psum", bufs=2, space="PSUM") as psum,
        ):
            # Internal DRAM buffers
            h_ag_in = dram.tile([shard, hidden], bf16)
            h_full = dram.tile([n_pad, hidden], bf16, addr_space="Shared")
            h2_ag_in = dram.tile([shard, ncls_pad], bf16)
            h2_full = dram.tile([n_pad, ncls_pad], bf16, addr_space="Shared")

            # Constants into SBUF
            w1_sb = const.tile([P, kt * hidden], bf16)
            nc.sync.dma_start(out=w1_sb[:], in_=w1_d[:])
            b1_sb = const.tile([P, 1], f32)
            nc.sync.dma_start(out=b1_sb[:], in_=b1_d[:])
            w2_sb = const.tile([hidden, ncls_pad], f32)
            nc.sync.dma_start(out=w2_sb[:], in_=w2_d[:])
            b2w_sb = const.tile([P, bpc * ncls_pad], f32)
            nc.sync.dma_start(out=b2w_sb[:], in_=b2w_d[:])
            iota_sb = const.tile([P, T * P], bf16)
            nc.sync.dma_start(out=iota_sb[:], in_=iota_d[:])
            srcs_sb = const.tile([P, nt], dt.int32)
            nc.sync.dma_start(out=srcs_sb[:], in_=srcs_d[:])
            segs_sb = const.tile([P, nt], bf16)
            nc.sync.dma_start(out=segs_sb[:], in_=segs_d[:])
            norms_sb = const.tile([P, nt], bf16)
            nc.sync.dma_start(out=norms_sb[:], in_=norms_d[:])
            dinvn_sb = const.tile([P, bpc], f32)
            nc.sync.dma_start(out=dinvn_sb[:], in_=dinvn_d[:])

            # Persistent big SBUF staging tiles
            h_big = sb.tile([P, bpc * hidden], bf16, tag="h_big", bufs=1)
            h2_big = sb.tile([P, bpc * ncls_pad], bf16, tag="h2_big", bufs=1)
            lg_big = sb.tile([P, bpc * ncls_pad], f32, tag="lg_big", bufs=1)
            expv = sb.tile([P, bpc * N_CLASSES], f32, tag="expv", bufs=1)
            out_big = sb.tile([P, bpc * N_CLASSES], f32, tag="out_big", bufs=1)
            maxs = sb.tile([P, bpc], f32, tag="maxs", bufs=1)
            sums = sb.tile([P, bpc], f32, tag="sums", bufs=1)
            lns = sb.tile([P, bpc], f32, tag="lns", bufs=1)

            iota3 = iota_sb[:].rearrange("p (t d) -> p t d", d=P)

            # ---------------- Phase 1: GEMM1 (h = x @ W1) ----------------
            with tc.tile_pool(name="xtp", bufs=1) as xtp:
                xt_sb = xtp.tile([P, kt * shard], bf16, tag="xt", bufs=1)
                for k in range(kt):
                    nc.sync.dma_start(
                        out=xt_sb[:, k * shard:(k + 1) * shard],
                        in_=xt_d[k * P:(k + 1) * P, :],
                    )
                for i in range(bpc):
                    psum_h = psum.tile([P, P], f32, tag="pmm")
                    for k in range(kt):
                        nc.tensor.matmul(
                            out=psum_h[:],
                            lhsT=xt_sb[:, k * shard + i * P:k * shard + (i + 1) * P],
                            rhs=w1_sb[:, k * hidden:(k + 1) * hidden],
                            start=(k == 0),
                            stop=(k == kt - 1),
                        )
                    nc.vector.tensor_copy(
                        out=h_big[:, i * hidden:(i + 1) * hidden], in_=psum_h[:]
                    )
                nc.sync.dma_start(
                    out=h_ag_in[:].rearrange("(t p) f -> p t f", p=P),
                    in_=h_big[:].rearrange("p (t f) -> p t f", f=hidden),
                )

            # ---------------- AllGather h ----------------
            if lvl >= 1:
                nc.gpsimd.collective_compute(
                    "AllGather",
                    mybir.AluOpType.bypass,
                    replica_groups=RG,
                    ins=[h_ag_in[:]],
                    outs=[h_full[:]],
                )

            # ---------------- Phase 2: Agg1 + relu + GEMM2 ----------------
            with tc.tile_pool(name="msgp", bufs=1) as msgp:
                for g in range(ngrp if lvl >= 2 else 0):
                    msg = msgp.tile([P, G * T * hidden], bf16, tag="msg", bufs=2)
                    nc.gpsimd.indirect_dma_start(
                        out=msg[:],
                        out_offset=None,
                        in_=h_full[:],
                        in_offset=bass.IndirectOffsetOnAxis(
                            ap=srcs_sb[:, g * G * T:(g + 1) * G * T], axis=0
                        ),
                    )
                    for j in range(G):
                        b = g * G + j
                        g0 = b * T
                        sel = sb.tile([P, T * P], bf16, tag="sel", bufs=4)
                        sel3 = sel[:].rearrange("p (t d) -> p t d", d=P)
                        nc.vector.tensor_tensor(
                            out=sel3,
                            in0=iota3,
                            in1=segs_sb[:, g0:g0 + T].to_broadcast([P, T, P]),
                            op=mybir.AluOpType.is_equal,
                        )
                        nc.vector.tensor_tensor(
                            out=sel3,
                            in0=sel3,
                            in1=norms_sb[:, g0:g0 + T].to_broadcast([P, T, P]),
                            op=mybir.AluOpType.mult,
                        )
                        psum1 = psum.tile([P, P], f32, tag="pmm")
                        for t in range(T):
                            nc.tensor.matmul(
                                out=psum1[:],
                                lhsT=msg[:, (j * T + t) * hidden:(j * T + t + 1) * hidden],
                                rhs=sel[:, t * P:(t + 1) * P],
                                start=(t == 0),
                                stop=(t == T - 1),
                            )
                        a1 = sb.tile([P, P], f32, tag="a1", bufs=3)
                        nc.scalar.activation(
                            out=a1[:], in_=psum1[:],
                            func=AF.Relu,
                            bias=b1_sb[:, 0:1],
                        )
                        psum2 = psum.tile([P, ncls_pad], f32, tag="pcl")
                        nc.tensor.matmul(
                            out=psum2[:], lhsT=a1[:], rhs=w2_sb[:],
                            start=True, stop=True,
                        )
                        # h2 row v scaled by dinv[v]: folds layer-2 src norm
                        # into the gather table.
                        nc.scalar.activation(
                            out=h2_big[:, b * ncls_pad:(b + 1) * ncls_pad],
                            in_=psum2[:],
                            func=AF.Copy,
                            scale=dinvn_sb[:, b:b + 1],
                        )
                if lvl >= 2:
                    nc.sync.dma_start(
                        out=h2_ag_in[:].rearrange("(t p) f -> p t f", p=P),
                        in_=h2_big[:].rearrange("p (t f) -> p t f", f=ncls_pad),
                    )

            # ---------------- AllGather h2 ----------------
            if lvl >= 3:
                nc.gpsimd.collective_compute(
                    "AllGather",
                    mybir.AluOpType.bypass,
                    replica_groups=RG,
                    ins=[h2_ag_in[:]],
                    outs=[h2_full[:]],
                )

            # ---------------- Phase 3: Agg2 ----------------
            with tc.tile_pool(name="msg2p", bufs=1) as msg2p:
                for g in range(ngrp if lvl >= 4 else 0):
                    msg2 = msg2p.tile([P, G * T * ncls_pad], bf16, tag="msg2",
                                      bufs=2)
                    nc.gpsimd.indirect_dma_start(
                        out=msg2[:],
                        out_offset=None,
                        in_=h2_full[:],
                        in_offset=bass.IndirectOffsetOnAxis(
                            ap=srcs_sb[:, g * G * T:(g + 1) * G * T], axis=0
                        ),
                    )
                    for j in range(G):
                        b = g * G + j
                        g0 = b * T
                        sel = sb.tile([P, T * P], bf16, tag="sel3", bufs=4)
                        sel3 = sel[:].rearrange("p (t d) -> p t d", d=P)
                        nc.vector.tensor_tensor(
                            out=sel3,
                            in0=iota3,
                            in1=segs_sb[:, g0:g0 + T].to_broadcast([P, T, P]),
                            op=mybir.AluOpType.is_equal,
                        )
                        psum_o = psum.tile([P, ncls_pad], f32, tag="pcl")
                        for t in range(T):
                            nc.tensor.matmul(
                                out=psum_o[:],
                                lhsT=sel[:, t * P:(t + 1) * P],
                                rhs=msg2[:, (j * T + t) * ncls_pad:(j * T + t + 1) * ncls_pad],
                                start=(t == 0),
                                stop=(t == T - 1),
                            )
                        nc.vector.tensor_scalar_mul(
                            out=lg_big[:, b * ncls_pad:(b + 1) * ncls_pad],
                            in0=psum_o[:],
                            scalar1=dinvn_sb[:, b:b + 1],
                        )

            # ------------- bias + batched log_softmax + store -------------
            if lvl >= 4:
                nc.vector.tensor_tensor(
                    out=lg_big[:], in0=lg_big[:], in1=b2w_sb[:],
                    op=mybir.AluOpType.add,
                )
                l40 = lg_big[:].rearrange(
                    "p (t f) -> p t f", f=ncls_pad)[:, :, 0:N_CLASSES]
                nc.vector.tensor_reduce(
                    out=maxs[:], in_=l40, axis=mybir.AxisListType.X,
                    op=mybir.AluOpType.max,
                )
                nc.vector.tensor_tensor(
                    out=l40, in0=l40,
                    in1=maxs[:].to_broadcast([P, bpc, N_CLASSES]),
                    op=mybir.AluOpType.subtract,
                )
                expv3 = expv[:].rearrange("p (t f) -> p t f", f=N_CLASSES)
                nc.scalar.activation(out=expv3, in_=l40, func=AF.Exp)
                nc.vector.tensor_reduce(
                    out=sums[:], in_=expv3, axis=mybir.AxisListType.X,
                    op=mybir.AluOpType.add,
                )
                nc.scalar.activation(out=lns[:], in_=sums[:], func=AF.Ln)
                nc.vector.tensor_tensor(
                    out=out_big[:].rearrange("p (t f) -> p t f", f=N_CLASSES),
                    in0=l40,
                    in1=lns[:].to_broadcast([P, bpc, N_CLASSES]),
                    op=mybir.AluOpType.subtract,
                )
                nc.sync.dma_start(
                    out=out_d[:].rearrange("(t p) f -> p t f", p=P),
                    in_=out_big[:].rearrange("p (t f) -> p t f", f=N_CLASSES),
                )

    nc.compile()
    return nc


# --------------------------------------------------------------------------
# Host orchestration
# --------------------------------------------------------------------------

def _prepare(x, edge_index, W1, b1, W2, b2, bpc):
    x = np.asarray(x, dtype=np.float32)
    W1 = np.asarray(W1, dtype=np.float32)
    b1v = np.asarray(b1, dtype=np.float32).reshape(-1)
    W2 = np.asarray(W2, dtype=np.float32)
    b2v = np.asarray(b2, dtype=np.float32).reshape(-1)

    n_nodes, f_in = x.shape
    hidden = W1.shape[1]
    ncls = W2.shape[1]
    assert hidden == P and ncls == N_CLASSES

    shard = bpc * P
    n_pad = NCORES * shard
    assert n_pad >= n_nodes

    srcs, segs, norms, dinv_pad, T = _preprocess(edge_index, n_nodes, bpc)

    nc = _build_program(f_in, hidden, NCLS_PAD, bpc, T)

    kt = f_in // P

    x_pad = np.zeros((n_pad, f_in), np.float32)
    x_pad[:n_nodes] = x
    w1r = np.ascontiguousarray(
        W1.reshape(kt, P, hidden).transpose(1, 0, 2).reshape(P, kt * hidden)
    ).astype(BF16)
    w2p = np.zeros((hidden, NCLS_PAD), np.float32)
    w2p[:, :ncls] = W2
    b2blk = np.zeros((P, NCLS_PAD), np.float32)
    b2blk[:, :ncls] = b2v[None, :]
    b2w = np.tile(b2blk, (1, bpc))
    iotaw = np.ascontiguousarray(
        np.broadcast_to(
            np.tile(np.arange(P, dtype=np.float32), T), (P, T * P)
        )
    ).astype(BF16)
    dinv_cores = dinv_pad.reshape(NCORES, bpc, P)

    in_maps = []
    for c in range(NCORES):
        xt_c = np.ascontiguousarray(
            x_pad[c * shard:(c + 1) * shard].T
        ).astype(BF16)
        in_maps.append({
            "xt": xt_c,
            "w1": w1r,
            "b1": b1v.reshape(P, 1).copy(),
            "w2": w2p,
            "b2w": b2w,
            "iotaw": iotaw,
            "srcs": np.ascontiguousarray(srcs[c]),
            "segs": np.ascontiguousarray(segs[c]),
            "norms": np.ascontiguousarray(norms[c]),
            "dinvn": np.ascontiguousarray(dinv_cores[c].T),
        })
    return nc, in_maps, T


def _run(x, edge_index, W1, b1, W2, b2, bpc):
    from concourse.bass_utils import run_bass_kernel_spmd

    global LAST_RESULT

    n_nodes = np.asarray(x).shape[0]
    nc, in_maps, T = _prepare(x, edge_index, W1, b1, W2, b2, bpc)

    res = run_bass_kernel_spmd(
        nc, in_maps, core_ids=list(range(NCORES)),
        trace=TRACE, trace_kwargs=dict(TRACE_KWARGS),
    )
    LAST_RESULT = {
        "exec_time_ns": res.exec_time_ns,
        "mean_exec_time_ns": res.mean_exec_time_ns,
        "instructions_and_trace": res.instructions_and_trace,
        "profile_json": res.profile_json,
        "T": T,
        "nc": nc,
        "in_maps": in_maps,
    }
    out = np.concatenate([r["out"] for r in res.results], axis=0)
    return out[:n_nodes]


def kernel(x, edge_index, W1, b1, W2, b2):
    n_nodes = np.asarray(x).shape[0]
    bpc = int(math.ceil(n_nodes / (NCORES * P)))
    return _run(x, edge_index, W1, b1, W2, b2, bpc)
